# revision 5
# baseline (speedup 1.0000x reference)
"""Trainium2 Bass kernel for nn_Net_27986006901491 (gnn_message_passing).

Reference computation (per branch, 3 layers):
    layer(eh) = Mg @ eh @ W.T + b        with leaky-relu between layers,
where Mg is a fixed 44x44 matrix determined by the graph:
    Mg = C @ B @ P,  P = diag(1/deg) A_dst^T,  B = A_dst^T A_src,
    C = 0.5 (A_src + A_dst)
(A_src/A_dst are the 44x14 one-hot edge->node incidence matrices.)

Strategy (8 cores, tensor-parallel on the 4096 output features):
 - Activations live transposed in SBUF: A^T = (Mg @ h)^T as [feat, edge]
   tiles [128, 32, 44+44].  The Mg multiply is computed as h^T @ Mg^T via
   lhsT=h, which produces exactly this transposed layout - no transposes
   anywhere.
 - Weights are pre-transposed on the host to W^T [4096, 512-slice] and
   split into bf16 hi/lo pairs; matmuls run 3 bf16 passes
   (hi*hi + hi*lo + lo*hi) accumulating in fp32 PSUM (~1e-5 rel err).
 - Biases are folded into the PSUM group as K=1 matmuls (ones x b_hi/lo).
 - Between layers each core all-gathers its 512-feature slice (packed
   hi|lo bf16, 90KB/rank) across the 8 cores.
 - Mg itself is built on device from one-hot incidence inputs (exact
   bf16 integer arithmetic; only 1/deg needs fp32, handled by a hi/lo
   split of diag(1/deg)H).
"""

import sys

sys.path.insert(0, "/opt/trn_rl_repo")

import ml_dtypes
import numpy as np

import concourse.bacc as bacc
import concourse.mybir as mybir
import concourse.tile as tile
from concourse import bass_utils

BF16 = ml_dtypes.bfloat16
P = 128
E = 44  # edges
NN = 14  # nodes
K = 4096
KO = K // P  # 32
NCORES = 8
NSL = K // NCORES  # 512 output features per core
F32 = mybir.dt.float32
BF = mybir.dt.bfloat16
Lrelu = mybir.ActivationFunctionType.Lrelu

_CACHE = {}


def _split_np(a):
    hi = a.astype(BF16)
    lo = (a.astype(np.float32) - hi.astype(np.float32)).astype(BF16)
    return hi, lo


def _build_nc():
    nc = bacc.Bacc("TRN2", target_bir_lowering=False, num_devices=NCORES)

    x_d = nc.dram_tensor("x", [E, K], F32, kind="ExternalInput")
    soh_d = nc.dram_tensor("soh", [E, NN], BF, kind="ExternalInput")
    doh_d = nc.dram_tensor("doh", [E, NN], BF, kind="ExternalInput")
    sth_d = nc.dram_tensor("sth", [NN, E], BF, kind="ExternalInput")
    dth_d = nc.dram_tensor("dth", [NN, E], BF, kind="ExternalInput")
    onesc_d = nc.dram_tensor("onesc", [E, 1], BF, kind="ExternalInput")
    ones1_d = nc.dram_tensor("ones1", [1, E], BF, kind="ExternalInput")

    LBS = ["a1", "v1", "a2", "v2", "a5"]
    w_d = {}
    for lb in LBS:
        w_d[lb] = (
            nc.dram_tensor(f"w{lb}h", [K, NSL], BF, kind="ExternalInput"),
            nc.dram_tensor(f"w{lb}l", [K, NSL], BF, kind="ExternalInput"),
            nc.dram_tensor(f"b{lb}h", [1, NSL], BF, kind="ExternalInput"),
            nc.dram_tensor(f"b{lb}l", [1, NSL], BF, kind="ExternalInput"),
        )
    w_d["v5"] = (
        nc.dram_tensor("wv5h", [K, 1], BF, kind="ExternalInput"),
        nc.dram_tensor("wv5l", [K, 1], BF, kind="ExternalInput"),
        nc.dram_tensor("bv5h", [1, 1], BF, kind="ExternalInput"),
        nc.dram_tensor("bv5l", [1, 1], BF, kind="ExternalInput"),
    )

    h1_d = nc.dram_tensor("h1", [E, NSL], F32, kind="ExternalOutput")
    h2_d = nc.dram_tensor("h2", [E, 1], F32, kind="ExternalOutput")

    with tile.TileContext(nc) as tc:
        with (
            tc.tile_pool(name="const", bufs=1) as const,
            tc.tile_pool(name="wp", bufs=2) as wp,
            tc.tile_pool(name="ap", bufs=3) as apool,
            tc.tile_pool(name="hp", bufs=2) as hp,
            tc.tile_pool(name="xp", bufs=2) as xp,
            tc.tile_pool(name="dramp", bufs=2, space="DRAM") as dramp,
            tc.tile_pool(name="mps", bufs=2, space="PSUM") as mps,
            tc.tile_pool(name="gps", bufs=4, space="PSUM") as gps,
            tc.tile_pool(name="sps", bufs=2, space="PSUM") as sps,
        ):
            # ---------- constants ----------
            soh = const.tile([E, NN], BF)
            nc.sync.dma_start(soh[:], soh_d[:])
            doh = const.tile([E, NN], BF)
            nc.sync.dma_start(doh[:], doh_d[:])
            sth = const.tile([NN, E], BF)
            nc.sync.dma_start(sth[:], sth_d[:])
            dth = const.tile([NN, E], BF)
            nc.sync.dma_start(dth[:], dth_d[:])
            onesc = const.tile([E, 1], BF)
            nc.sync.dma_start(onesc[:], onesc_d[:])
            ones1 = const.tile([1, E], BF)
            nc.sync.dma_start(ones1[:], ones1_d[:])

            SPMAX = [P, KO * E]  # largest split temp (fp32 [128, 1408])

            def split_f32(src_ap, hi_ap, lo_ap, shape, tag):
                """hi = bf16(src); lo = bf16(src - fp32(hi)). src fp32 AP."""
                n = int(np.prod(shape[1:]))
                nc.vector.tensor_copy(hi_ap, src_ap)
                up = hp.tile(SPMAX, F32, name=f"up_{tag}", tag="sp32")
                upv = up[: shape[0], :n].rearrange(
                    "p (a b) -> p a b", a=shape[1]
                ) if len(shape) == 3 else up[: shape[0], :n]
                nc.vector.tensor_copy(upv, hi_ap)
                df = hp.tile(SPMAX, F32, name=f"df_{tag}", tag="sp32")
                dfv = df[: shape[0], :n].rearrange(
                    "p (a b) -> p a b", a=shape[1]
                ) if len(shape) == 3 else df[: shape[0], :n]
                nc.vector.tensor_tensor(dfv, src_ap, upv, mybir.AluOpType.subtract)
                nc.vector.tensor_copy(lo_ap, dfv)

            # ---------- build MgT on device ----------
            # deg = A_dst^T @ ones ;  r = 1/deg
            ps_deg = sps.tile([NN, 1], F32, name="ps_deg", tag="sps")
            nc.tensor.matmul(ps_deg[:], doh[:], onesc[:], start=True, stop=True)
            r_t = const.tile([NN, 1], F32)
            nc.vector.reciprocal(r_t[:], ps_deg[:])

            # Gt = D^T S  (as lhsT for H this indexes as G[i,j])
            ps_gt = sps.tile([NN, NN], F32, name="ps_gt", tag="sps")
            nc.tensor.matmul(ps_gt[:], doh[:], soh[:], start=True, stop=True)
            gt_bf = const.tile([NN, NN], BF)
            nc.vector.tensor_copy(gt_bf[:], ps_gt[:])

            # CT = 0.5 (S^T + D^T)   (exact in bf16: values in {0, .5, 1})
            ct_bf = const.tile([NN, E], BF)
            nc.vector.tensor_tensor(ct_bf[:], sth[:], dth[:], mybir.AluOpType.add)
            nc.vector.tensor_scalar_mul(ct_bf[:], ct_bf[:], 0.5)

            # H = G @ CT  [14, 44]
            ps_h = sps.tile([NN, E], F32, name="ps_h", tag="sps")
            nc.tensor.matmul(ps_h[:], gt_bf[:], ct_bf[:], start=True, stop=True)
            rh = const.tile([NN, E], F32)
            nc.vector.tensor_scalar_mul(rh[:], ps_h[:], r_t[:])
            rh_hi = const.tile([NN, E], BF)
            rh_lo = const.tile([NN, E], BF)
            split_f32(rh[:], rh_hi[:], rh_lo[:], (NN, E), "rh")

            # MgT = D @ diag(r) @ H : lhsT = D^T (one-hot, exact bf16)
            ps_m = sps.tile([E, E], F32, name="ps_m", tag="sps")
            nc.tensor.matmul(ps_m[:], dth[:], rh_hi[:], start=True, stop=False)
            nc.tensor.matmul(ps_m[:], dth[:], rh_lo[:], start=False, stop=True)
            mgt_f = const.tile([E, E], F32)
            nc.vector.tensor_copy(mgt_f[:], ps_m[:])
            mgt_hi = const.tile([E, E], BF)
            mgt_lo = const.tile([E, E], BF)
            split_f32(mgt_f[:], mgt_hi[:], mgt_lo[:], (E, E), "mgt")

            # ---------- A0 = (Mg @ x)^T as [128, 32, 44hi | 44lo] ----------
            a0 = apool.tile([P, KO, 2 * E], BF, name="a0", tag="a_0")
            a0f = const.tile([P, KO, E], F32)
            XP = 8  # x pieces of 512 cols
            for pi in range(XP):
                xs = xp.tile([E, K // XP], F32, name=f"x_{pi}", tag="x")
                nc.sync.dma_start(xs[:], x_d[:, pi * (K // XP) : (pi + 1) * (K // XP)])
                xhi = xp.tile([E, K // XP], BF, name=f"xhi_{pi}", tag="xhi")
                xlo = xp.tile([E, K // XP], BF, name=f"xlo_{pi}", tag="xlo")
                split_f32(xs[:], xhi[:], xlo[:], (E, K // XP), f"x{pi}")
                for kj in range(K // XP // P):
                    ko = pi * (K // XP // P) + kj
                    pg = gps.tile([P, E], F32, name=f"a0g_{ko}", tag="gps")
                    sl = slice(kj * P, (kj + 1) * P)
                    nc.tensor.matmul(pg[:], xhi[:, sl], mgt_hi[:], start=True, stop=False)
                    nc.tensor.matmul(pg[:], xhi[:, sl], mgt_lo[:], start=False, stop=False)
                    nc.tensor.matmul(pg[:], xlo[:, sl], mgt_hi[:], start=False, stop=True)
                    nc.vector.tensor_copy(a0f[:, ko, :], pg[:])
            split_f32(
                a0f[:], a0[:, :, :E], a0[:, :, E:], (P, KO, E), "a0"
            )

            # ---------- one tensor-parallel GCN layer ----------
            CHUNKS = 8  # weight DMA chunks per hi/lo tensor (4 k-tiles each)
            KPC = KO // CHUNKS  # 4
            WTAGS = 16  # sliding ring of weight-chunk slots (4KB each)
            widx = [0]

            def wchunk(lb, half, ci):
                i = widx[0]
                widx[0] += 1
                return wp.tile(
                    [P, KPC, NSL], BF, name=f"w{half}{ci}_{lb}",
                    tag=f"w{i % WTAGS}", bufs=1,
                )

            def layer(a_t, lb, n_out, act, mg_ag, out_dram, a_tag):
                wh_d, wl_d, bh_d, bl_d = w_d[lb]
                whs, wls = [], []
                for ci in range(CHUNKS):
                    rows = slice(ci * KPC * P, (ci + 1) * KPC * P)
                    wh = wchunk(lb, "h", ci)
                    nc.sync.dma_start(
                        wh[:, :, :n_out],
                        wh_d[rows, :].rearrange("(ko ki) n -> ki ko n", ki=P),
                    )
                    whs.append(wh)
                    wl = wchunk(lb, "l", ci)
                    nc.sync.dma_start(
                        wl[:, :, :n_out],
                        wl_d[rows, :].rearrange("(ko ki) n -> ki ko n", ki=P),
                    )
                    wls.append(wl)
                bh = hp.tile([1, NSL], BF, name=f"bh_{lb}", tag="bh", bufs=1)
                nc.sync.dma_start(bh[:, :n_out], bh_d[:])
                bl = hp.tile([1, NSL], BF, name=f"bl_{lb}", tag="bl", bufs=1)
                nc.sync.dma_start(bl[:, :n_out], bl_d[:])

                a_hi = a_t[:, :, :E]
                a_lo = a_t[:, :, E:]
                psum = mps.tile([E, NSL], F32, name=f"ps_{lb}", tag="mps")
                pso = psum[:, :n_out]
                for ci in range(CHUNKS):
                    for kj in range(KPC):
                        ko = ci * KPC + kj
                        first = ko == 0
                        nc.tensor.matmul(
                            pso, a_hi[:, ko, :], whs[ci][:, kj, :n_out],
                            start=first, stop=False,
                        )
                        nc.tensor.matmul(
                            pso, a_hi[:, ko, :], wls[ci][:, kj, :n_out],
                            start=False, stop=False,
                        )
                        nc.tensor.matmul(
                            pso, a_lo[:, ko, :], whs[ci][:, kj, :n_out],
                            start=False, stop=False,
                        )
                nc.tensor.matmul(pso, ones1[:], bh[:, :n_out], start=False, stop=False)
                nc.tensor.matmul(pso, ones1[:], bl[:, :n_out], start=False, stop=True)

                if not act:
                    ho = hp.tile([E, NSL], F32, name=f"ho_{lb}", tag="h")
                    nc.vector.tensor_copy(ho[:, :n_out], pso)
                    nc.sync.dma_start(out_dram[:], ho[:, :n_out])
                    return None

                h = hp.tile([E, NSL], F32, name=f"h_{lb}", tag="h")
                nc.scalar.activation(h[:], psum[:], Lrelu, alpha=0.01)
                if not mg_ag:
                    return None

                # Mg multiply (transposed out) + hi/lo pack + AllGather
                hhi = hp.tile([E, NSL], BF, name=f"hhi_{lb}", tag="hsplit")
                hlo = hp.tile([E, NSL], BF, name=f"hlo_{lb}", tag="hsplit")
                split_f32(h[:], hhi[:], hlo[:], (E, NSL), f"h{lb}")
                mgf = hp.tile([P, NSL // P, E], F32, name=f"mgf_{lb}", tag="mgf")
                for fo in range(NSL // P):
                    pg = gps.tile([P, E], F32, name=f"mg_{lb}_{fo}", tag="gps")
                    sl = slice(fo * P, (fo + 1) * P)
                    nc.tensor.matmul(pg[:], hhi[:, sl], mgt_hi[:], start=True, stop=False)
                    nc.tensor.matmul(pg[:], hhi[:, sl], mgt_lo[:], start=False, stop=False)
                    nc.tensor.matmul(pg[:], hlo[:, sl], mgt_hi[:], start=False, stop=True)
                    nc.vector.tensor_copy(mgf[:, fo, :], pg[:])
                agin = hp.tile([P, NSL // P, 2 * E], BF, name=f"agin_{lb}", tag="agin")
                split_f32(
                    mgf[:], agin[:, :, :E], agin[:, :, E:], (P, NSL // P, E), f"ag{lb}"
                )
                bin_t = dramp.tile([NSL, 2 * E], BF, name=f"bin_{lb}", tag="bin")
                nc.sync.dma_start(
                    bin_t[:].rearrange("(fo ki) c -> ki fo c", ki=P), agin[:]
                )
                bout_t = dramp.tile(
                    [K, 2 * E], BF, name=f"bout_{lb}", tag="bout", addr_space="Shared"
                )
                nc.gpsimd.collective_compute(
                    "AllGather",
                    mybir.AluOpType.bypass,
                    replica_groups=[list(range(NCORES))],
                    ins=[bin_t.opt()],
                    outs=[bout_t.opt()],
                )
                a_next = apool.tile([P, KO, 2 * E], BF, name=f"a_{lb}", tag=a_tag)
                nc.sync.dma_start(
                    a_next[:], bout_t[:].rearrange("(ko ki) c -> ki ko c", ki=P)
                )
                return a_next

            a_a1 = layer(a0, "a1", NSL, True, True, None, "a_1")
            a_v1 = layer(a0, "v1", NSL, True, True, None, "a_2")
            a_a2 = layer(a_a1, "a2", NSL, True, True, None, "a_0")
            a_v2 = layer(a_v1, "v2", NSL, True, True, None, "a_1")
            layer(a_a2, "a5", NSL, False, False, h1_d, None)
            layer(a_v2, "v5", 1, False, False, h2_d, None)

    nc.compile()
    return nc


def _prepare_inputs(x, edge_src, edge_dst, weights):
    """Per-core input maps. weights = dict of Wa1..bv5 numpy arrays."""
    src = np.asarray(edge_src)
    dst = np.asarray(edge_dst)
    ar = np.arange(NN)
    soh = (src[:, None] == ar[None, :]).astype(BF16)
    doh = (dst[:, None] == ar[None, :]).astype(BF16)
    sth = np.ascontiguousarray(soh.T)
    dth = np.ascontiguousarray(doh.T)

    common = {
        "x": np.ascontiguousarray(np.asarray(x, np.float32)),
        "soh": soh,
        "doh": doh,
        "sth": sth,
        "dth": dth,
        "onesc": np.ones((E, 1), BF16),
        "ones1": np.ones((1, E), BF16),
    }

    # v5 (replicated)
    wt5 = np.ascontiguousarray(np.asarray(weights["Wv5"], np.float32).T)  # [4096, 1]
    w5h, w5l = _split_np(wt5)
    b5 = np.asarray(weights["bv5"], np.float32).reshape(1, 1)
    b5h, b5l = _split_np(b5)
    common.update({"wv5h": w5h, "wv5l": w5l, "bv5h": b5h, "bv5l": b5l})

    per_core = [dict(common) for _ in range(NCORES)]
    name_map = {"a1": "Wa1", "a2": "Wa2", "a5": "Wa5", "v1": "Wv1", "v2": "Wv2"}
    bias_map = {"a1": "ba1", "a2": "ba2", "a5": "ba5", "v1": "bv1", "v2": "bv2"}
    for lb, wname in name_map.items():
        wt = np.ascontiguousarray(np.asarray(weights[wname], np.float32).T)  # [in, out]
        b = np.asarray(weights[bias_map[lb]], np.float32)
        for c in range(NCORES):
            sl = np.ascontiguousarray(wt[:, c * NSL : (c + 1) * NSL])
            hi, lo = _split_np(sl)
            bh, bl = _split_np(b[c * NSL : (c + 1) * NSL].reshape(1, NSL))
            per_core[c][f"w{lb}h"] = hi
            per_core[c][f"w{lb}l"] = lo
            per_core[c][f"b{lb}h"] = bh
            per_core[c][f"b{lb}l"] = bl
    return per_core


def _run(in_maps, **kwargs):
    if "nc" not in _CACHE:
        _CACHE["nc"] = _build_nc()
    return bass_utils.run_bass_kernel_spmd(
        _CACHE["nc"], in_maps, core_ids=list(range(NCORES)), **kwargs
    )


def kernel(
    x, edge_src, edge_dst,
    Wa1, ba1, Wa2, ba2, Wa5, ba5,
    Wv1, bv1, Wv2, bv2, Wv5, bv5,
    **run_kwargs,
):
    weights = {
        "Wa1": Wa1, "ba1": ba1, "Wa2": Wa2, "ba2": ba2, "Wa5": Wa5, "ba5": ba5,
        "Wv1": Wv1, "bv1": bv1, "Wv2": Wv2, "bv2": bv2, "Wv5": Wv5, "bv5": bv5,
    }
    in_maps = _prepare_inputs(x, edge_src, edge_dst, weights)
    res = _run(in_maps, **run_kwargs)
    h1 = np.concatenate([res.results[c]["h1"] for c in range(NCORES)], axis=1)
    h2 = res.results[0]["h2"]
    kernel.last_result = res
    return (h1, h2)


# revision 7
# speedup vs baseline: 1.1752x; 1.1752x over previous
"""Trainium2 Bass kernel for nn_Net_27986006901491 (gnn_message_passing).

Reference computation (per branch, 3 layers):
    layer(eh) = Mg @ eh @ W.T + b        with leaky-relu between layers,
where Mg is a fixed 44x44 matrix determined by the graph:
    Mg = C @ B @ P,  P = diag(1/deg) A_dst^T,  B = A_dst^T A_src,
    C = 0.5 (A_src + A_dst)
(A_src/A_dst are the 44x14 one-hot edge->node incidence matrices.)

Strategy (8 cores, tensor-parallel on the 4096 hidden features):
 - Activations live transposed in SBUF: A^T = (Mg @ h)^T as [feat, edge]
   tiles.  The Mg multiply is computed as h^T @ Mg^T via lhsT=h, which
   produces exactly this transposed layout - no transposes anywhere.
 - Weights are pre-transposed/bf16-hi/lo-split/packed on the host into
   fully contiguous 1MB DMA chunks (8KB per partition row); matmuls run
   3 bf16 passes (hi*hi + hi*lo + lo*hi) into fp32 PSUM (~1e-5 rel err).
 - Biases fold into the PSUM group as K=1 matmuls (ones x b_hi/lo).
 - Layer 1 and 2: column-parallel (each core produces a 512-feature
   slice), followed by an AllGather of the Mg-transposed activations
   (packed hi|lo bf16, 90KB/rank).
 - Layer 3 (Wa5/Wv5): row-parallel partial sums over each core's local
   512-feature slice - NO collective; the host sums the 8 partial
   outputs (bias is supplied to core 0 only).
 - Mg itself is built on device from one-hot incidence inputs (exact
   bf16 integer arithmetic; only 1/deg needs fp32, via a hi/lo split).
"""

import sys

sys.path.insert(0, "/opt/trn_rl_repo")

import ml_dtypes
import numpy as np

import concourse.bacc as bacc
import concourse.mybir as mybir
import concourse.tile as tile
from concourse import bass_utils

BF16 = ml_dtypes.bfloat16
P = 128
E = 44  # edges
NN = 14  # nodes
K = 4096
KO = K // P  # 32
NCORES = 8
NSL = K // NCORES  # 512 features per core
F32 = mybir.dt.float32
BF = mybir.dt.bfloat16
Lrelu = mybir.ActivationFunctionType.Lrelu

_CACHE = {}


def _split_np(a):
    hi = a.astype(BF16)
    lo = (a.astype(np.float32) - hi.astype(np.float32)).astype(BF16)
    return hi, lo


def _pack_chunks(wt):
    """wt [rows, cols] fp32 -> packed bf16 [rows, 2*cols] laid out so that
    DMA chunk ci = packed[ci*128:(ci+1)*128, :] is one contiguous block with
    partition ki holding, for each k-subtile kt, [hi | lo] of row
    (ci*KT + kt)*128 + ki.  rows must be a multiple of 512."""
    rows, cols = wt.shape
    KT = 4  # k-subtiles per chunk
    nch = rows // (KT * P)
    hi, lo = _split_np(wt)
    hi4 = hi.reshape(nch, KT, P, cols)
    lo4 = lo.reshape(nch, KT, P, cols)
    pk = np.concatenate([hi4, lo4], axis=3)  # [nch, KT, P, 2*cols]
    pk = pk.transpose(0, 2, 1, 3).reshape(nch * P, KT * 2 * cols)
    return np.ascontiguousarray(pk)


def _build_nc():
    nc = bacc.Bacc("TRN2", target_bir_lowering=False, num_devices=NCORES)

    x_d = nc.dram_tensor("x", [E, K], F32, kind="ExternalInput")
    soh_d = nc.dram_tensor("soh", [E, NN], BF, kind="ExternalInput")
    doh_d = nc.dram_tensor("doh", [E, NN], BF, kind="ExternalInput")
    sth_d = nc.dram_tensor("sth", [NN, E], BF, kind="ExternalInput")
    dth_d = nc.dram_tensor("dth", [NN, E], BF, kind="ExternalInput")
    onesc_d = nc.dram_tensor("onesc", [E, 1], BF, kind="ExternalInput")
    ones1_d = nc.dram_tensor("ones1", [1, E], BF, kind="ExternalInput")

    # packed weights: column-parallel layers [8*128, 4*1024]; row-parallel
    # final layers a5 [8*128, 4*1024] (nt-major), v5 [128, 8]
    w_d, b_d = {}, {}
    for lb in ["a1", "v1", "a2", "v2", "a5"]:
        w_d[lb] = nc.dram_tensor(f"w{lb}", [8 * P, 4096], BF, kind="ExternalInput")
        b_d[lb] = nc.dram_tensor(
            f"b{lb}", [1, 8 * 1024 if lb == "a5" else 1024], BF, kind="ExternalInput"
        )
    w_d["v5"] = nc.dram_tensor("wv5", [P, 8], BF, kind="ExternalInput")
    b_d["v5"] = nc.dram_tensor("bv5", [1, 2], BF, kind="ExternalInput")

    h1_d = nc.dram_tensor("h1", [E, K], F32, kind="ExternalOutput")
    h2_d = nc.dram_tensor("h2", [E, 1], F32, kind="ExternalOutput")

    with tile.TileContext(nc) as tc:
        with (
            tc.tile_pool(name="const", bufs=1) as const,
            tc.tile_pool(name="wp", bufs=1) as wp,
            tc.tile_pool(name="ap", bufs=1) as apool,
            tc.tile_pool(name="hp", bufs=2) as hp,
            tc.tile_pool(name="xp", bufs=2) as xp,
            tc.tile_pool(name="dramp", bufs=2, space="DRAM") as dramp,
            tc.tile_pool(name="mps", bufs=2, space="PSUM") as mps,
            tc.tile_pool(name="gps", bufs=4, space="PSUM") as gps,
            tc.tile_pool(name="sps", bufs=2, space="PSUM") as sps,
        ):
            # ---------- constants ----------
            soh = const.tile([E, NN], BF)
            nc.sync.dma_start(soh[:], soh_d[:])
            doh = const.tile([E, NN], BF)
            nc.sync.dma_start(doh[:], doh_d[:])
            sth = const.tile([NN, E], BF)
            nc.sync.dma_start(sth[:], sth_d[:])
            dth = const.tile([NN, E], BF)
            nc.sync.dma_start(dth[:], dth_d[:])
            onesc = const.tile([E, 1], BF)
            nc.sync.dma_start(onesc[:], onesc_d[:])
            ones1 = const.tile([1, E], BF)
            nc.sync.dma_start(ones1[:], ones1_d[:])

            SPMAX = [P, 16 * E]  # split temp (fp32 [128, 704])

            def split_f32(src_ap, hi_ap, lo_ap, shape, tag):
                """hi = bf16(src); lo = bf16(src - fp32(hi)). src fp32 AP."""
                n = int(np.prod(shape[1:]))
                nc.vector.tensor_copy(hi_ap, src_ap)
                up = hp.tile(SPMAX, F32, name=f"up_{tag}", tag="sp32")
                upv = (
                    up[: shape[0], :n].rearrange("p (a b) -> p a b", a=shape[1])
                    if len(shape) == 3
                    else up[: shape[0], :n]
                )
                nc.vector.tensor_copy(upv, hi_ap)
                df = hp.tile(SPMAX, F32, name=f"df_{tag}", tag="sp32")
                dfv = (
                    df[: shape[0], :n].rearrange("p (a b) -> p a b", a=shape[1])
                    if len(shape) == 3
                    else df[: shape[0], :n]
                )
                nc.vector.tensor_tensor(dfv, src_ap, upv, mybir.AluOpType.subtract)
                nc.vector.tensor_copy(lo_ap, dfv)

            # ---------- build MgT on device ----------
            ps_deg = sps.tile([NN, 1], F32, name="ps_deg", tag="sps")
            nc.tensor.matmul(ps_deg[:], doh[:], onesc[:], start=True, stop=True)
            r_t = const.tile([NN, 1], F32)
            nc.vector.reciprocal(r_t[:], ps_deg[:])

            ps_gt = sps.tile([NN, NN], F32, name="ps_gt", tag="sps")
            nc.tensor.matmul(ps_gt[:], doh[:], soh[:], start=True, stop=True)
            gt_bf = const.tile([NN, NN], BF)
            nc.vector.tensor_copy(gt_bf[:], ps_gt[:])

            ct_bf = const.tile([NN, E], BF)
            nc.vector.tensor_tensor(ct_bf[:], sth[:], dth[:], mybir.AluOpType.add)
            nc.vector.tensor_scalar_mul(ct_bf[:], ct_bf[:], 0.5)

            ps_h = sps.tile([NN, E], F32, name="ps_h", tag="sps")
            nc.tensor.matmul(ps_h[:], gt_bf[:], ct_bf[:], start=True, stop=True)
            rh = const.tile([NN, E], F32)
            nc.vector.tensor_scalar_mul(rh[:], ps_h[:], r_t[:])
            rh_hi = const.tile([NN, E], BF)
            rh_lo = const.tile([NN, E], BF)
            split_f32(rh[:], rh_hi[:], rh_lo[:], (NN, E), "rh")

            ps_m = sps.tile([E, E], F32, name="ps_m", tag="sps")
            nc.tensor.matmul(ps_m[:], dth[:], rh_hi[:], start=True, stop=False)
            nc.tensor.matmul(ps_m[:], dth[:], rh_lo[:], start=False, stop=True)
            mgt_f = const.tile([E, E], F32)
            nc.vector.tensor_copy(mgt_f[:], ps_m[:])
            mgt_hi = const.tile([E, E], BF)
            mgt_lo = const.tile([E, E], BF)
            split_f32(mgt_f[:], mgt_hi[:], mgt_lo[:], (E, E), "mgt")

            # ---------- A0 = (Mg @ x)^T as [128, 32, 44hi | 44lo] ----------
            a0 = apool.tile([P, KO, 2 * E], BF, name="a0", tag="a_0")
            a0f = const.tile([P, KO, E], F32)
            XP = 8  # x pieces of 512 cols
            for pi in range(XP):
                xs = xp.tile([E, K // XP], F32, name=f"x_{pi}", tag="x")
                nc.sync.dma_start(xs[:], x_d[:, pi * (K // XP) : (pi + 1) * (K // XP)])
                xhi = xp.tile([E, K // XP], BF, name=f"xhi_{pi}", tag="xhi")
                xlo = xp.tile([E, K // XP], BF, name=f"xlo_{pi}", tag="xlo")
                split_f32(xs[:], xhi[:], xlo[:], (E, K // XP), f"x{pi}")
                for kj in range(K // XP // P):
                    ko = pi * (K // XP // P) + kj
                    pg = gps.tile([P, E], F32, name=f"a0g_{ko}", tag="gps")
                    sl = slice(kj * P, (kj + 1) * P)
                    nc.tensor.matmul(pg[:], xhi[:, sl], mgt_hi[:], start=True, stop=False)
                    nc.tensor.matmul(pg[:], xhi[:, sl], mgt_lo[:], start=False, stop=False)
                    nc.tensor.matmul(pg[:], xlo[:, sl], mgt_hi[:], start=False, stop=True)
                    nc.vector.tensor_copy(a0f[:, ko, :], pg[:])
            for hf in range(2):  # split in halves to keep the temp small
                sl = slice(hf * (KO // 2), (hf + 1) * (KO // 2))
                split_f32(
                    a0f[:, sl, :], a0[:, sl, :E], a0[:, sl, E:],
                    (P, KO // 2, E), f"a0{hf}",
                )

            # ---------- weight chunk ring ----------
            WTAGS = 16
            widx = [0]

            def wchunk(name):
                i = widx[0]
                widx[0] += 1
                return wp.tile(
                    [P, 4096], BF, name=name, tag=f"w{i % WTAGS}", bufs=1
                )

            def brow(name, src_ap, n):
                t = hp.tile([1, 1024], BF, name=name, tag="brow", bufs=2)
                nc.sync.dma_start(t[:, :n], src_ap)
                return t

            def mg_pack(h, lb, dst):
                """dst [P, 4, 2E] <- hi/lo of (Mg h)^T for this core's slice."""
                hhi = hp.tile([E, NSL], BF, name=f"hhi_{lb}", tag="hsplit")
                hlo = hp.tile([E, NSL], BF, name=f"hlo_{lb}", tag="hsplit")
                split_f32(h[:], hhi[:], hlo[:], (E, NSL), f"h{lb}")
                mgf = hp.tile([P, NSL // P, E], F32, name=f"mgf_{lb}", tag="mgf")
                for fo in range(NSL // P):
                    pg = gps.tile([P, E], F32, name=f"mg_{lb}_{fo}", tag="gps")
                    sl = slice(fo * P, (fo + 1) * P)
                    nc.tensor.matmul(pg[:], hhi[:, sl], mgt_hi[:], start=True, stop=False)
                    nc.tensor.matmul(pg[:], hhi[:, sl], mgt_lo[:], start=False, stop=False)
                    nc.tensor.matmul(pg[:], hlo[:, sl], mgt_hi[:], start=False, stop=True)
                    nc.vector.tensor_copy(mgf[:, fo, :], pg[:])
                split_f32(
                    mgf[:], dst[:, :, :E], dst[:, :, E:], (P, NSL // P, E), f"ag{lb}"
                )

            # ---------- column-parallel layer (L1 / L2) ----------
            def layer(a_t, lb, mg_mode, a_tag=None):
                a_hi = a_t[:, :, :E]
                a_lo = a_t[:, :, E:]
                bh = brow(f"b_{lb}", b_d[lb][:], 1024)
                psum = mps.tile([E, NSL], F32, name=f"ps_{lb}", tag="mps")
                for ci in range(8):
                    wc = wchunk(f"w_{lb}_{ci}")
                    nc.sync.dma_start(wc[:], w_d[lb][ci * P : (ci + 1) * P, :])
                    for kt in range(4):
                        ko = ci * 4 + kt
                        base = kt * 1024
                        nc.tensor.matmul(
                            psum[:], a_hi[:, ko, :], wc[:, base : base + NSL],
                            start=(ko == 0), stop=False,
                        )
                        nc.tensor.matmul(
                            psum[:], a_hi[:, ko, :], wc[:, base + NSL : base + 1024],
                            start=False, stop=False,
                        )
                        nc.tensor.matmul(
                            psum[:], a_lo[:, ko, :], wc[:, base : base + NSL],
                            start=False, stop=False,
                        )
                nc.tensor.matmul(psum[:], ones1[:], bh[:, :NSL], start=False, stop=False)
                nc.tensor.matmul(psum[:], ones1[:], bh[:, NSL:1024], start=False, stop=True)

                h = hp.tile([E, NSL], F32, name=f"h_{lb}", tag="h")
                nc.scalar.activation(h[:], psum[:], Lrelu, alpha=0.01)

                if mg_mode == "local":
                    mgloc = hp.tile(
                        [P, NSL // P, 2 * E], BF, name=f"mgloc_{lb}", tag="mgloc"
                    )
                    mg_pack(h, lb, mgloc)
                    return mgloc

                agin = hp.tile([P, NSL // P, 2 * E], BF, name=f"agin_{lb}", tag="agin")
                mg_pack(h, lb, agin)
                bin_t = dramp.tile([NSL, 2 * E], BF, name=f"bin_{lb}", tag="bin")
                nc.sync.dma_start(
                    bin_t[:].rearrange("(fo ki) c -> ki fo c", ki=P), agin[:]
                )
                bout_t = dramp.tile(
                    [K, 2 * E], BF, name=f"bout_{lb}", tag="bout", addr_space="Shared"
                )
                nc.gpsimd.collective_compute(
                    "AllGather",
                    mybir.AluOpType.bypass,
                    replica_groups=[list(range(NCORES))],
                    ins=[bin_t.opt()],
                    outs=[bout_t.opt()],
                )
                a_next = apool.tile([P, KO, 2 * E], BF, name=f"a_{lb}", tag=a_tag)
                nc.sync.dma_start(
                    a_next[:], bout_t[:].rearrange("(ko ki) c -> ki ko c", ki=P)
                )
                return a_next

            # ---------- row-parallel partial final layer (no collective) ----
            def partial_layer(mgloc, lb, NT, n_out, out_dram):
                m_hi = mgloc[:, :, :E]
                m_lo = mgloc[:, :, E:]
                if lb == "v5":
                    wc = wchunk(f"w_{lb}")
                    nc.sync.dma_start(wc[:, :8], w_d[lb][:])
                for nt in range(NT):
                    if lb != "v5":
                        wc = wchunk(f"w_{lb}_{nt}")
                        nc.sync.dma_start(wc[:], w_d[lb][nt * P : (nt + 1) * P, :])
                    bh = brow(
                        f"b_{lb}_{nt}",
                        b_d[lb][:, nt * 2 * n_out : (nt + 1) * 2 * n_out],
                        2 * n_out,
                    )
                    psum = mps.tile([E, NSL], F32, name=f"ps_{lb}_{nt}", tag="mps")
                    pso = psum[:, :n_out]
                    for kj in range(4):
                        base = kj * 2 * n_out
                        nc.tensor.matmul(
                            pso, m_hi[:, kj, :], wc[:, base : base + n_out],
                            start=(kj == 0), stop=False,
                        )
                        nc.tensor.matmul(
                            pso, m_hi[:, kj, :], wc[:, base + n_out : base + 2 * n_out],
                            start=False, stop=False,
                        )
                        nc.tensor.matmul(
                            pso, m_lo[:, kj, :], wc[:, base : base + n_out],
                            start=False, stop=False,
                        )
                    nc.tensor.matmul(pso, ones1[:], bh[:, :n_out], start=False, stop=False)
                    nc.tensor.matmul(
                        pso, ones1[:], bh[:, n_out : 2 * n_out], start=False, stop=True
                    )
                    ho = hp.tile([E, NSL], F32, name=f"ho_{lb}_{nt}", tag="h")
                    nc.vector.tensor_copy(ho[:, :n_out], pso)
                    nc.sync.dma_start(
                        out_dram[:, nt * n_out : (nt + 1) * n_out], ho[:, :n_out]
                    )

            a_a1 = layer(a0, "a1", "ag", "a_1")
            a_v1 = layer(a0, "v1", "ag", "a_2")
            mg_a2 = layer(a_a1, "a2", "local")
            mg_v2 = layer(a_v1, "v2", "local")
            partial_layer(mg_a2, "a5", 8, NSL, h1_d)
            partial_layer(mg_v2, "v5", 1, 1, h2_d)

    nc.compile()
    return nc


def _prepare_inputs(x, edge_src, edge_dst, weights):
    src = np.asarray(edge_src)
    dst = np.asarray(edge_dst)
    ar = np.arange(NN)
    soh = (src[:, None] == ar[None, :]).astype(BF16)
    doh = (dst[:, None] == ar[None, :]).astype(BF16)

    common = {
        "x": np.ascontiguousarray(np.asarray(x, np.float32)),
        "soh": soh,
        "doh": doh,
        "sth": np.ascontiguousarray(soh.T),
        "dth": np.ascontiguousarray(doh.T),
        "onesc": np.ones((E, 1), BF16),
        "ones1": np.ones((1, E), BF16),
    }
    per_core = [dict(common) for _ in range(NCORES)]

    # column-parallel layers: core c gets W^T[:, c*512:(c+1)*512] packed
    for lb, wn, bn in [
        ("a1", "Wa1", "ba1"), ("v1", "Wv1", "bv1"),
        ("a2", "Wa2", "ba2"), ("v2", "Wv2", "bv2"),
    ]:
        wt = np.asarray(weights[wn], np.float32).T  # [in, out]
        b = np.asarray(weights[bn], np.float32)
        for c in range(NCORES):
            sl = np.ascontiguousarray(wt[:, c * NSL : (c + 1) * NSL])
            per_core[c][f"w{lb}"] = _pack_chunks(sl)
            bh, bl = _split_np(b[c * NSL : (c + 1) * NSL].reshape(1, NSL))
            per_core[c][f"b{lb}"] = np.ascontiguousarray(
                np.concatenate([bh, bl], axis=1)
            )

    # a5: row-parallel. core c gets rows c*512..(c+1)*512 of Wa5^T, packed
    # nt-major: chunk nt covers output cols nt*512..(nt+1)*512.
    wt5 = np.asarray(weights["Wa5"], np.float32).T  # [4096, 4096]
    b5 = np.asarray(weights["ba5"], np.float32)
    b5h, b5l = _split_np(b5.reshape(8, NSL))
    b5pk = np.concatenate([b5h, b5l], axis=1).reshape(1, 8 * 1024)  # nt-major hi|lo
    z5 = np.zeros_like(b5pk)
    for c in range(NCORES):
        rows = wt5[c * NSL : (c + 1) * NSL, :]  # [512, 4096]
        # build [8 nt-chunks, 128 rows, 4 kj, 1024] packing
        hi, lo = _split_np(rows)
        hi4 = hi.reshape(4, P, 8, NSL).transpose(2, 1, 0, 3)  # [nt, ki, kj, 512]
        lo4 = lo.reshape(4, P, 8, NSL).transpose(2, 1, 0, 3)
        pk = np.concatenate([hi4, lo4], axis=3)  # [nt, ki, kj, 1024]
        per_core[c]["wa5"] = np.ascontiguousarray(pk.reshape(8 * P, 4096))
        per_core[c]["ba5"] = b5pk if c == 0 else z5

    # v5: rows c*512.. of Wv5^T [4096, 1] -> [128, 8] (kj-major hi|lo pairs)
    wv5t = np.asarray(weights["Wv5"], np.float32).T  # [4096, 1]
    bv5 = np.asarray(weights["bv5"], np.float32).reshape(1, 1)
    bvh, bvl = _split_np(bv5)
    bv5pk = np.concatenate([bvh, bvl], axis=1)
    zv = np.zeros_like(bv5pk)
    for c in range(NCORES):
        rows = wv5t[c * NSL : (c + 1) * NSL, 0]  # [512]
        hi, lo = _split_np(rows)
        pk = np.stack([hi.reshape(4, P), lo.reshape(4, P)], axis=2)  # [kj, ki, 2]
        per_core[c]["wv5"] = np.ascontiguousarray(
            pk.transpose(1, 0, 2).reshape(P, 8)
        )
        per_core[c]["bv5"] = bv5pk if c == 0 else zv
    return per_core


def _run(in_maps, **kwargs):
    if "nc" not in _CACHE:
        _CACHE["nc"] = _build_nc()
    return bass_utils.run_bass_kernel_spmd(
        _CACHE["nc"], in_maps, core_ids=list(range(NCORES)), **kwargs
    )


def kernel(
    x, edge_src, edge_dst,
    Wa1, ba1, Wa2, ba2, Wa5, ba5,
    Wv1, bv1, Wv2, bv2, Wv5, bv5,
    **run_kwargs,
):
    weights = {
        "Wa1": Wa1, "ba1": ba1, "Wa2": Wa2, "ba2": ba2, "Wa5": Wa5, "ba5": ba5,
        "Wv1": Wv1, "bv1": bv1, "Wv2": Wv2, "bv2": bv2, "Wv5": Wv5, "bv5": bv5,
    }
    in_maps = _prepare_inputs(x, edge_src, edge_dst, weights)
    res = _run(in_maps, **run_kwargs)
    h1 = np.sum([res.results[c]["h1"] for c in range(NCORES)], axis=0, dtype=np.float64)
    h2 = np.sum([res.results[c]["h2"] for c in range(NCORES)], axis=0, dtype=np.float64)
    kernel.last_result = res
    return (h1.astype(np.float32), h2.astype(np.float32))


# revision 9
# speedup vs baseline: 1.2069x; 1.0269x over previous
"""Trainium2 Bass kernel for nn_Net_27986006901491 (gnn_message_passing).

Reference computation (per branch, 3 layers):
    layer(eh) = Mg @ eh @ W.T + b        with leaky-relu between layers,
where Mg is a fixed 44x44 matrix determined by the graph:
    Mg = C @ B @ P,  P = diag(1/deg) A_dst^T,  B = A_dst^T A_src,
    C = 0.5 (A_src + A_dst)
(A_src/A_dst are the 44x14 one-hot edge->node incidence matrices.)

Strategy (8 cores, tensor-parallel on the 4096 hidden features):
 - Activations live transposed in SBUF: A^T = (Mg @ h)^T as [feat, edge]
   tiles.  The Mg multiply is computed as h^T @ Mg^T via lhsT=h, which
   produces exactly this transposed layout - no transposes anywhere.
 - Weights are pre-transposed/bf16-hi/lo-split/packed on the host into
   fully contiguous 1MB DMA chunks (8KB per partition row); matmuls run
   3 bf16 passes (hi*hi + hi*lo + lo*hi) into fp32 PSUM (~1e-5 rel err).
 - Biases fold into the PSUM group as K=1 matmuls (ones x b_hi/lo).
 - Layer 1 and 2: column-parallel (each core produces a 512-feature
   slice), followed by an AllGather of the Mg-transposed activations
   (packed hi|lo bf16, 90KB/rank).
 - Layer 3 (Wa5/Wv5): row-parallel partial sums over each core's local
   512-feature slice - NO collective; the host sums the 8 partial
   outputs (bias is supplied to core 0 only).
 - Mg itself is built on device from one-hot incidence inputs (exact
   bf16 integer arithmetic; only 1/deg needs fp32, via a hi/lo split).
"""

import sys

sys.path.insert(0, "/opt/trn_rl_repo")

import ml_dtypes
import numpy as np

import concourse.bacc as bacc
import concourse.mybir as mybir
import concourse.tile as tile
from concourse import bass_utils

BF16 = ml_dtypes.bfloat16
P = 128
E = 44  # edges
NN = 14  # nodes
K = 4096
KO = K // P  # 32
NCORES = 8
NSL = K // NCORES  # 512 features per core
F32 = mybir.dt.float32
BF = mybir.dt.bfloat16
Lrelu = mybir.ActivationFunctionType.Lrelu

_CACHE = {}


def _split_np(a):
    hi = a.astype(BF16)
    lo = (a.astype(np.float32) - hi.astype(np.float32)).astype(BF16)
    return hi, lo


def _pack_chunks(wt):
    """wt [rows, cols] fp32 -> packed bf16 [rows, 2*cols] laid out so that
    DMA chunk ci = packed[ci*128:(ci+1)*128, :] is one contiguous block with
    partition ki holding, for each k-subtile kt, [hi | lo] of row
    (ci*KT + kt)*128 + ki.  rows must be a multiple of 512."""
    rows, cols = wt.shape
    KT = 4  # k-subtiles per chunk
    nch = rows // (KT * P)
    hi, lo = _split_np(wt)
    hi4 = hi.reshape(nch, KT, P, cols)
    lo4 = lo.reshape(nch, KT, P, cols)
    pk = np.concatenate([hi4, lo4], axis=3)  # [nch, KT, P, 2*cols]
    pk = pk.transpose(0, 2, 1, 3).reshape(nch * P, KT * 2 * cols)
    return np.ascontiguousarray(pk)


def _build_nc():
    nc = bacc.Bacc("TRN2", target_bir_lowering=False, num_devices=NCORES)

    x_d = nc.dram_tensor("x", [E, K], F32, kind="ExternalInput")
    soh_d = nc.dram_tensor("soh", [E, NN], BF, kind="ExternalInput")
    doh_d = nc.dram_tensor("doh", [E, NN], BF, kind="ExternalInput")
    sth_d = nc.dram_tensor("sth", [NN, E], BF, kind="ExternalInput")
    dth_d = nc.dram_tensor("dth", [NN, E], BF, kind="ExternalInput")
    onesc_d = nc.dram_tensor("onesc", [E, 1], BF, kind="ExternalInput")
    ones1_d = nc.dram_tensor("ones1", [1, E], BF, kind="ExternalInput")

    # packed weights: column-parallel layers [8*128, 4*1024]; row-parallel
    # final layers a5 [8*128, 4*1024] (nt-major), v5 [128, 8]
    w_d, b_d = {}, {}
    for lb in ["a1", "v1", "a2", "v2", "a5"]:
        w_d[lb] = nc.dram_tensor(f"w{lb}", [8 * P, 4096], BF, kind="ExternalInput")
        b_d[lb] = nc.dram_tensor(
            f"b{lb}", [1, 8 * 1024 if lb == "a5" else 1024], BF, kind="ExternalInput"
        )
    w_d["v5"] = nc.dram_tensor("wv5", [P, 8], BF, kind="ExternalInput")
    b_d["v5"] = nc.dram_tensor("bv5", [1, 2], BF, kind="ExternalInput")

    h1_d = nc.dram_tensor("h1", [E, K], F32, kind="ExternalOutput")
    h2_d = nc.dram_tensor("h2", [E, 1], F32, kind="ExternalOutput")

    with tile.TileContext(nc) as tc:
        with (
            tc.tile_pool(name="const", bufs=1) as const,
            tc.tile_pool(name="wp", bufs=1) as wp,
            tc.tile_pool(name="ap", bufs=1) as apool,
            tc.tile_pool(name="hp", bufs=2) as hp,
            tc.tile_pool(name="xp", bufs=2) as xp,
            tc.tile_pool(name="dramp", bufs=2, space="DRAM") as dramp,
            tc.tile_pool(name="mps", bufs=2, space="PSUM") as mps,
            tc.tile_pool(name="gps", bufs=4, space="PSUM") as gps,
            tc.tile_pool(name="sps", bufs=2, space="PSUM") as sps,
        ):
            # ---------- constants ----------
            soh = const.tile([E, NN], BF)
            nc.sync.dma_start(soh[:], soh_d[:])
            doh = const.tile([E, NN], BF)
            nc.sync.dma_start(doh[:], doh_d[:])
            sth = const.tile([NN, E], BF)
            nc.sync.dma_start(sth[:], sth_d[:])
            dth = const.tile([NN, E], BF)
            nc.sync.dma_start(dth[:], dth_d[:])
            onesc = const.tile([E, 1], BF)
            nc.sync.dma_start(onesc[:], onesc_d[:])
            ones1 = const.tile([1, E], BF)
            nc.sync.dma_start(ones1[:], ones1_d[:])

            SPMAX = [P, 16 * E]  # split temp (fp32 [128, 704])

            def split_f32(src_ap, hi_ap, lo_ap, shape, tag):
                """hi = bf16(src); lo = bf16(src - fp32(hi)). src fp32 AP."""
                n = int(np.prod(shape[1:]))
                nc.vector.tensor_copy(hi_ap, src_ap)
                up = hp.tile(SPMAX, F32, name=f"up_{tag}", tag="sp32")
                upv = (
                    up[: shape[0], :n].rearrange("p (a b) -> p a b", a=shape[1])
                    if len(shape) == 3
                    else up[: shape[0], :n]
                )
                nc.vector.tensor_copy(upv, hi_ap)
                df = hp.tile(SPMAX, F32, name=f"df_{tag}", tag="sp32")
                dfv = (
                    df[: shape[0], :n].rearrange("p (a b) -> p a b", a=shape[1])
                    if len(shape) == 3
                    else df[: shape[0], :n]
                )
                nc.vector.tensor_tensor(dfv, src_ap, upv, mybir.AluOpType.subtract)
                nc.vector.tensor_copy(lo_ap, dfv)

            # ---------- build MgT on device ----------
            ps_deg = sps.tile([NN, 1], F32, name="ps_deg", tag="sps")
            nc.tensor.matmul(ps_deg[:], doh[:], onesc[:], start=True, stop=True)
            r_t = const.tile([NN, 1], F32)
            nc.vector.reciprocal(r_t[:], ps_deg[:])

            ps_gt = sps.tile([NN, NN], F32, name="ps_gt", tag="sps")
            nc.tensor.matmul(ps_gt[:], doh[:], soh[:], start=True, stop=True)
            gt_bf = const.tile([NN, NN], BF)
            nc.vector.tensor_copy(gt_bf[:], ps_gt[:])

            ct_bf = const.tile([NN, E], BF)
            nc.vector.tensor_tensor(ct_bf[:], sth[:], dth[:], mybir.AluOpType.add)
            nc.vector.tensor_scalar_mul(ct_bf[:], ct_bf[:], 0.5)

            ps_h = sps.tile([NN, E], F32, name="ps_h", tag="sps")
            nc.tensor.matmul(ps_h[:], gt_bf[:], ct_bf[:], start=True, stop=True)
            rh = const.tile([NN, E], F32)
            nc.vector.tensor_scalar_mul(rh[:], ps_h[:], r_t[:])
            rh_hi = const.tile([NN, E], BF)
            rh_lo = const.tile([NN, E], BF)
            split_f32(rh[:], rh_hi[:], rh_lo[:], (NN, E), "rh")

            ps_m = sps.tile([E, E], F32, name="ps_m", tag="sps")
            nc.tensor.matmul(ps_m[:], dth[:], rh_hi[:], start=True, stop=False)
            nc.tensor.matmul(ps_m[:], dth[:], rh_lo[:], start=False, stop=True)
            mgt_f = const.tile([E, E], F32)
            nc.vector.tensor_copy(mgt_f[:], ps_m[:])
            mgt_hi = const.tile([E, E], BF)
            mgt_lo = const.tile([E, E], BF)
            split_f32(mgt_f[:], mgt_hi[:], mgt_lo[:], (E, E), "mgt")

            # ---------- A0 = (Mg @ x)^T as [128, 32, 44hi | 44lo] ----------
            a0 = apool.tile([P, KO, 2 * E], BF, name="a0", tag="a_0")
            a0f = const.tile([P, KO, E], F32)
            XP = 8  # x pieces of 512 cols
            for pi in range(XP):
                xs = xp.tile([E, K // XP], F32, name=f"x_{pi}", tag="x")
                nc.sync.dma_start(xs[:], x_d[:, pi * (K // XP) : (pi + 1) * (K // XP)])
                xhi = xp.tile([E, K // XP], BF, name=f"xhi_{pi}", tag="xhi")
                xlo = xp.tile([E, K // XP], BF, name=f"xlo_{pi}", tag="xlo")
                split_f32(xs[:], xhi[:], xlo[:], (E, K // XP), f"x{pi}")
                for kj in range(K // XP // P):
                    ko = pi * (K // XP // P) + kj
                    pg = gps.tile([P, E], F32, name=f"a0g_{ko}", tag="gps")
                    sl = slice(kj * P, (kj + 1) * P)
                    nc.tensor.matmul(pg[:], xhi[:, sl], mgt_hi[:], start=True, stop=False)
                    nc.tensor.matmul(pg[:], xhi[:, sl], mgt_lo[:], start=False, stop=False)
                    nc.tensor.matmul(pg[:], xlo[:, sl], mgt_hi[:], start=False, stop=True)
                    nc.vector.tensor_copy(a0f[:, ko, :], pg[:])
            for hf in range(2):  # split in halves to keep the temp small
                sl = slice(hf * (KO // 2), (hf + 1) * (KO // 2))
                split_f32(
                    a0f[:, sl, :], a0[:, sl, :E], a0[:, sl, E:],
                    (P, KO // 2, E), f"a0{hf}",
                )

            # ---------- weight chunk ring ----------
            WTAGS = 16
            widx = [0]

            def wchunk(name):
                i = widx[0]
                widx[0] += 1
                return wp.tile(
                    [P, 4096], BF, name=name, tag=f"w{i % WTAGS}", bufs=1
                )

            def brow(name, src_ap, n):
                t = hp.tile([1, 1024], BF, name=name, tag="brow", bufs=2)
                nc.scalar.dma_start(t[:, :n], src_ap)
                return t

            def mg_pack(h, lb, dst):
                """dst [P, 4, 2E] <- hi/lo of (Mg h)^T for this core's slice."""
                hhi = hp.tile([E, NSL], BF, name=f"hhi_{lb}", tag="hsplit")
                hlo = hp.tile([E, NSL], BF, name=f"hlo_{lb}", tag="hsplit")
                split_f32(h[:], hhi[:], hlo[:], (E, NSL), f"h{lb}")
                mgf = hp.tile([P, NSL // P, E], F32, name=f"mgf_{lb}", tag="mgf")
                for fo in range(NSL // P):
                    pg = gps.tile([P, E], F32, name=f"mg_{lb}_{fo}", tag="gps")
                    sl = slice(fo * P, (fo + 1) * P)
                    nc.tensor.matmul(pg[:], hhi[:, sl], mgt_hi[:], start=True, stop=False)
                    nc.tensor.matmul(pg[:], hhi[:, sl], mgt_lo[:], start=False, stop=False)
                    nc.tensor.matmul(pg[:], hlo[:, sl], mgt_hi[:], start=False, stop=True)
                    nc.vector.tensor_copy(mgf[:, fo, :], pg[:])
                split_f32(
                    mgf[:], dst[:, :, :E], dst[:, :, E:], (P, NSL // P, E), f"ag{lb}"
                )

            # ---------- column-parallel layer (L1 / L2) ----------
            def layer(a_t, lb, mg_mode, a_tag=None):
                a_hi = a_t[:, :, :E]
                a_lo = a_t[:, :, E:]
                bh = brow(f"b_{lb}", b_d[lb][:], 1024)
                psum = mps.tile([E, NSL], F32, name=f"ps_{lb}", tag="mps")
                for ci in range(8):
                    wc = wchunk(f"w_{lb}_{ci}")
                    nc.sync.dma_start(wc[:], w_d[lb][ci * P : (ci + 1) * P, :])
                    for kt in range(4):
                        ko = ci * 4 + kt
                        base = kt * 1024
                        nc.tensor.matmul(
                            psum[:], a_hi[:, ko, :], wc[:, base : base + NSL],
                            start=(ko == 0), stop=False,
                        )
                        nc.tensor.matmul(
                            psum[:], a_hi[:, ko, :], wc[:, base + NSL : base + 1024],
                            start=False, stop=False,
                        )
                        nc.tensor.matmul(
                            psum[:], a_lo[:, ko, :], wc[:, base : base + NSL],
                            start=False, stop=False,
                        )
                nc.tensor.matmul(psum[:], ones1[:], bh[:, :NSL], start=False, stop=False)
                nc.tensor.matmul(psum[:], ones1[:], bh[:, NSL:1024], start=False, stop=True)

                h = hp.tile([E, NSL], F32, name=f"h_{lb}", tag="h")
                nc.scalar.activation(h[:], psum[:], Lrelu, alpha=0.01)

                if mg_mode == "local":
                    mgloc = hp.tile(
                        [P, NSL // P, 2 * E], BF, name=f"mgloc_{lb}", tag="mgloc"
                    )
                    mg_pack(h, lb, mgloc)
                    return mgloc

                agin = hp.tile([P, NSL // P, 2 * E], BF, name=f"agin_{lb}", tag="agin")
                mg_pack(h, lb, agin)
                bin_t = dramp.tile([NSL, 2 * E], BF, name=f"bin_{lb}", tag="bin")
                nc.gpsimd.dma_start(
                    bin_t[:].rearrange("(fo ki) c -> ki fo c", ki=P), agin[:]
                )
                bout_t = dramp.tile(
                    [K, 2 * E], BF, name=f"bout_{lb}", tag="bout", addr_space="Shared"
                )
                nc.gpsimd.collective_compute(
                    "AllGather",
                    mybir.AluOpType.bypass,
                    replica_groups=[list(range(NCORES))],
                    ins=[bin_t.opt()],
                    outs=[bout_t.opt()],
                )
                a_next = apool.tile([P, KO, 2 * E], BF, name=f"a_{lb}", tag=a_tag)
                nc.gpsimd.dma_start(
                    a_next[:], bout_t[:].rearrange("(ko ki) c -> ki ko c", ki=P)
                )
                return a_next

            # ---------- row-parallel partial final layer (no collective) ----
            def partial_layer(mgloc, lb, NT, n_out, out_dram):
                m_hi = mgloc[:, :, :E]
                m_lo = mgloc[:, :, E:]
                if lb == "v5":
                    wc = wchunk(f"w_{lb}")
                    nc.sync.dma_start(wc[:, :8], w_d[lb][:])
                for nt in range(NT):
                    if lb != "v5":
                        wc = wchunk(f"w_{lb}_{nt}")
                        nc.sync.dma_start(wc[:], w_d[lb][nt * P : (nt + 1) * P, :])
                    bh = brow(
                        f"b_{lb}_{nt}",
                        b_d[lb][:, nt * 2 * n_out : (nt + 1) * 2 * n_out],
                        2 * n_out,
                    )
                    psum = mps.tile([E, NSL], F32, name=f"ps_{lb}_{nt}", tag="mps")
                    pso = psum[:, :n_out]
                    for kj in range(4):
                        base = kj * 2 * n_out
                        nc.tensor.matmul(
                            pso, m_hi[:, kj, :], wc[:, base : base + n_out],
                            start=(kj == 0), stop=False,
                        )
                        nc.tensor.matmul(
                            pso, m_hi[:, kj, :], wc[:, base + n_out : base + 2 * n_out],
                            start=False, stop=False,
                        )
                        nc.tensor.matmul(
                            pso, m_lo[:, kj, :], wc[:, base : base + n_out],
                            start=False, stop=False,
                        )
                    nc.tensor.matmul(pso, ones1[:], bh[:, :n_out], start=False, stop=False)
                    nc.tensor.matmul(
                        pso, ones1[:], bh[:, n_out : 2 * n_out], start=False, stop=True
                    )
                    ho = hp.tile([E, NSL], F32, name=f"ho_{lb}_{nt}", tag="h")
                    nc.vector.tensor_copy(ho[:, :n_out], pso)
                    nc.scalar.dma_start(
                        out_dram[:, nt * n_out : (nt + 1) * n_out], ho[:, :n_out]
                    )

            a_a1 = layer(a0, "a1", "ag", "a_1")
            a_v1 = layer(a0, "v1", "ag", "a_2")
            mg_a2 = layer(a_a1, "a2", "local")
            partial_layer(mg_a2, "a5", 8, NSL, h1_d)
            mg_v2 = layer(a_v1, "v2", "local")
            partial_layer(mg_v2, "v5", 1, 1, h2_d)

    nc.compile()
    return nc


def _prepare_inputs(x, edge_src, edge_dst, weights):
    src = np.asarray(edge_src)
    dst = np.asarray(edge_dst)
    ar = np.arange(NN)
    soh = (src[:, None] == ar[None, :]).astype(BF16)
    doh = (dst[:, None] == ar[None, :]).astype(BF16)

    common = {
        "x": np.ascontiguousarray(np.asarray(x, np.float32)),
        "soh": soh,
        "doh": doh,
        "sth": np.ascontiguousarray(soh.T),
        "dth": np.ascontiguousarray(doh.T),
        "onesc": np.ones((E, 1), BF16),
        "ones1": np.ones((1, E), BF16),
    }
    per_core = [dict(common) for _ in range(NCORES)]

    # column-parallel layers: core c gets W^T[:, c*512:(c+1)*512] packed
    for lb, wn, bn in [
        ("a1", "Wa1", "ba1"), ("v1", "Wv1", "bv1"),
        ("a2", "Wa2", "ba2"), ("v2", "Wv2", "bv2"),
    ]:
        wt = np.asarray(weights[wn], np.float32).T  # [in, out]
        b = np.asarray(weights[bn], np.float32)
        for c in range(NCORES):
            sl = np.ascontiguousarray(wt[:, c * NSL : (c + 1) * NSL])
            per_core[c][f"w{lb}"] = _pack_chunks(sl)
            bh, bl = _split_np(b[c * NSL : (c + 1) * NSL].reshape(1, NSL))
            per_core[c][f"b{lb}"] = np.ascontiguousarray(
                np.concatenate([bh, bl], axis=1)
            )

    # a5: row-parallel. core c gets rows c*512..(c+1)*512 of Wa5^T, packed
    # nt-major: chunk nt covers output cols nt*512..(nt+1)*512.
    wt5 = np.asarray(weights["Wa5"], np.float32).T  # [4096, 4096]
    b5 = np.asarray(weights["ba5"], np.float32)
    b5h, b5l = _split_np(b5.reshape(8, NSL))
    b5pk = np.concatenate([b5h, b5l], axis=1).reshape(1, 8 * 1024)  # nt-major hi|lo
    z5 = np.zeros_like(b5pk)
    for c in range(NCORES):
        rows = wt5[c * NSL : (c + 1) * NSL, :]  # [512, 4096]
        # build [8 nt-chunks, 128 rows, 4 kj, 1024] packing
        hi, lo = _split_np(rows)
        hi4 = hi.reshape(4, P, 8, NSL).transpose(2, 1, 0, 3)  # [nt, ki, kj, 512]
        lo4 = lo.reshape(4, P, 8, NSL).transpose(2, 1, 0, 3)
        pk = np.concatenate([hi4, lo4], axis=3)  # [nt, ki, kj, 1024]
        per_core[c]["wa5"] = np.ascontiguousarray(pk.reshape(8 * P, 4096))
        per_core[c]["ba5"] = b5pk if c == 0 else z5

    # v5: rows c*512.. of Wv5^T [4096, 1] -> [128, 8] (kj-major hi|lo pairs)
    wv5t = np.asarray(weights["Wv5"], np.float32).T  # [4096, 1]
    bv5 = np.asarray(weights["bv5"], np.float32).reshape(1, 1)
    bvh, bvl = _split_np(bv5)
    bv5pk = np.concatenate([bvh, bvl], axis=1)
    zv = np.zeros_like(bv5pk)
    for c in range(NCORES):
        rows = wv5t[c * NSL : (c + 1) * NSL, 0]  # [512]
        hi, lo = _split_np(rows)
        pk = np.stack([hi.reshape(4, P), lo.reshape(4, P)], axis=2)  # [kj, ki, 2]
        per_core[c]["wv5"] = np.ascontiguousarray(
            pk.transpose(1, 0, 2).reshape(P, 8)
        )
        per_core[c]["bv5"] = bv5pk if c == 0 else zv
    return per_core


def _run(in_maps, **kwargs):
    if "nc" not in _CACHE:
        _CACHE["nc"] = _build_nc()
    return bass_utils.run_bass_kernel_spmd(
        _CACHE["nc"], in_maps, core_ids=list(range(NCORES)), **kwargs
    )


def kernel(
    x, edge_src, edge_dst,
    Wa1, ba1, Wa2, ba2, Wa5, ba5,
    Wv1, bv1, Wv2, bv2, Wv5, bv5,
    **run_kwargs,
):
    weights = {
        "Wa1": Wa1, "ba1": ba1, "Wa2": Wa2, "ba2": ba2, "Wa5": Wa5, "ba5": ba5,
        "Wv1": Wv1, "bv1": bv1, "Wv2": Wv2, "bv2": bv2, "Wv5": Wv5, "bv5": bv5,
    }
    in_maps = _prepare_inputs(x, edge_src, edge_dst, weights)
    res = _run(in_maps, **run_kwargs)
    h1 = np.sum([res.results[c]["h1"] for c in range(NCORES)], axis=0, dtype=np.float64)
    h2 = np.sum([res.results[c]["h2"] for c in range(NCORES)], axis=0, dtype=np.float64)
    kernel.last_result = res
    return (h1.astype(np.float32), h2.astype(np.float32))
